# revision 1
# baseline (speedup 1.0000x reference)
"""Trainium2 Bass kernel: NF4 (bitsandbytes-style) dequant + linear.

y = x @ dequant(weight_q, absmax).T + bias

x:        [4, 2048, 4096] f32
weight_q: [11008, 4096] int32 (values 0..15, NF4 codes)
absmax:   [11008, 64] f32 (per-64-block scales)
bias:     [11008] f32
out:      [4, 2048, 11008] f32

Sharding: column-parallel over out_features across 8 cores (1376 each).
x is replicated (host-transposed to [4096, 8192] so the contraction dim
lands on SBUF partitions). Per core the kernel:
  1. dequantizes its weight slice to bf16 [i, o] tiles via a DVE
     select-tree on the 4 NF4 index bits, scaled by absmax,
  2. streams x^T tiles, casts f32->bf16 on the scalar engine,
  3. accumulates y[s, o] = sum_k xT[k, s].T @ wT[k, o] in PSUM (f32),
  4. adds bias on DVE and DMAs the f32 result out.
Host concatenates the 8 column slices.
"""

import numpy as np

import concourse.bacc as bacc
import concourse.mybir as mybir
import concourse.tile as tile
from concourse.alu_op_type import AluOpType
from concourse.bass_utils import run_bass_kernel_spmd

DT = mybir.dt

NF4 = [
    -1.0, -0.6961928009986877, -0.5250730514526367, -0.39491748809814453,
    -0.28444138169288635, -0.18477343022823334, -0.09105003625154495, 0.0,
    0.07958029955625534, 0.16093020141124725, 0.24611230194568634,
    0.33791524171829224, 0.44070982933044434, 0.5626170039176941,
    0.7229568362236023, 1.0]

P = 128
IN_F = 4096
OUT_F = 11008
N_CORES = 8
O_LOC = OUT_F // N_CORES          # 1376 out features per core
S_TOT = 4 * 2048                  # 8192 tokens
KT = IN_F // P                    # 32 contraction tiles
ST = S_TOT // P                   # 64 token tiles
O_CHUNKS = [(0, 512), (512, 512), (1024, 352)]

_CACHE = {}


def _dequant_tile(nc, dq, wpool, q_t, sc_t, oi, kt, osz):
    """Emit DVE ops converting a [128, osz] uint16 NF4-index tile into a
    bf16 weight tile scaled by sc_t. Returns the persistent weight tile."""
    b0 = dq.tile([P, osz], DT.uint16, tag="b0")
    nc.vector.tensor_scalar(b0[:], q_t[:], 1, None, AluOpType.bitwise_and)
    m1 = dq.tile([P, osz], DT.uint16, tag="m1")
    nc.vector.tensor_scalar(m1[:], q_t[:], 1, 1,
                            AluOpType.logical_shift_right, AluOpType.bitwise_and)
    m2 = dq.tile([P, osz], DT.uint16, tag="m2")
    nc.vector.tensor_scalar(m2[:], q_t[:], 2, 1,
                            AluOpType.logical_shift_right, AluOpType.bitwise_and)
    m3 = dq.tile([P, osz], DT.uint16, tag="m3")
    nc.vector.tensor_scalar(m3[:], q_t[:], 3, None, AluOpType.logical_shift_right)

    t2 = []
    for j in range(4):
        t2_j = dq.tile([P, osz], DT.bfloat16, tag=f"t2_{j}")
        nc.vector.tensor_scalar(t2_j[:], b0[:], NF4[4 * j + 1] - NF4[4 * j],
                                NF4[4 * j], AluOpType.mult, AluOpType.add)
        o2_j = dq.tile([P, osz], DT.bfloat16, tag=f"o2_{j}")
        nc.vector.tensor_scalar(o2_j[:], b0[:], NF4[4 * j + 3] - NF4[4 * j + 2],
                                NF4[4 * j + 2], AluOpType.mult, AluOpType.add)
        nc.vector.copy_predicated(t2_j[:], m1[:], o2_j[:])
        t2.append(t2_j)

    t3 = []
    for j in range(2):
        t3_j = dq.tile([P, osz], DT.bfloat16, tag=f"t3_{j}")
        nc.vector.tensor_copy(t3_j[:], t2[2 * j][:])
        nc.vector.copy_predicated(t3_j[:], m2[:], t2[2 * j + 1][:])
        t3.append(t3_j)

    t4 = dq.tile([P, osz], DT.bfloat16, tag="t4")
    nc.vector.tensor_copy(t4[:], t3[0][:])
    nc.vector.copy_predicated(t4[:], m3[:], t3[1][:])

    w_t = wpool.tile([P, osz], DT.bfloat16, tag=f"w_{oi}_{kt}")
    nc.vector.tensor_tensor(w_t[:], t4[:], sc_t[:], AluOpType.mult)
    return w_t


def _build():
    nc = bacc.Bacc()
    xT = nc.dram_tensor("xT", [IN_F, S_TOT], DT.float32, kind="ExternalInput")
    qT = nc.dram_tensor("qT", [IN_F, O_LOC], DT.uint16, kind="ExternalInput")
    scale = nc.dram_tensor("scale", [KT, P, O_LOC], DT.float32, kind="ExternalInput")
    biasb = nc.dram_tensor("biasb", [P, O_LOC], DT.float32, kind="ExternalInput")
    y = nc.dram_tensor("y", [S_TOT, O_LOC], DT.float32, kind="ExternalOutput")

    with tile.TileContext(nc) as tc:
        with (
            tc.tile_pool(name="w", bufs=1) as wpool,
            tc.tile_pool(name="dq", bufs=2) as dq,
            tc.tile_pool(name="x", bufs=2) as xp,
            tc.tile_pool(name="o", bufs=4) as op,
            tc.tile_pool(name="ps", bufs=4, space="PSUM") as psp,
            tc.tile_pool(name="c", bufs=1) as cst,
        ):
            bias_t = cst.tile([P, O_LOC], DT.float32)
            nc.sync.dma_start(out=bias_t[:], in_=biasb[:])

            # ---- dequant all local weights (oi-major so matmuls can start
            # as soon as the first o-chunk's 32 k-tiles are ready) ----
            wt = {}
            for oi, (o0, osz) in enumerate(O_CHUNKS):
                for kt in range(KT):
                    q_t = dq.tile([P, osz], DT.uint16, tag="q")
                    nc.sync.dma_start(out=q_t[:],
                                      in_=qT[kt * P:(kt + 1) * P, o0:o0 + osz])
                    sc_t = dq.tile([P, osz], DT.float32, tag="sc")
                    nc.sync.dma_start(out=sc_t[:], in_=scale[kt, :, o0:o0 + osz])
                    wt[(oi, kt)] = _dequant_tile(nc, dq, wpool, q_t, sc_t, oi, kt, osz)

            # ---- main loop over token tiles ----
            for st in range(ST):
                s0 = st * P
                xb = xp.tile([P, KT, P], DT.bfloat16, tag="xb")
                for h in range(2):  # two halves of the k range
                    xf = xp.tile([P, KT // 2, P], DT.float32, tag=f"xf{h}")
                    for g in range(4):  # 4 DMAs per half
                        kt0 = h * 16 + g * 4
                        nc.sync.dma_start(
                            out=xf[:, g * 4:(g + 1) * 4, :],
                            in_=xT[kt0 * P:(kt0 + 4) * P, s0:s0 + P]
                                .rearrange("(k p) s -> p k s", p=P))
                    nc.scalar.copy(out=xb[:, h * 16:(h + 1) * 16, :], in_=xf[:])

                for oi, (o0, osz) in enumerate(O_CHUNKS):
                    ps_t = psp.tile([P, osz], DT.float32, tag="ps")
                    for kt in range(KT):
                        nc.tensor.matmul(ps_t[:], xb[:, kt, :], wt[(oi, kt)][:],
                                         start=(kt == 0), stop=(kt == KT - 1))
                    out_t = op.tile([P, osz], DT.float32, tag="out")
                    nc.vector.tensor_tensor(out_t[:], ps_t[:],
                                            bias_t[:, o0:o0 + osz], AluOpType.add)
                    nc.sync.dma_start(out=y[s0:s0 + P, o0:o0 + osz], in_=out_t[:])

    nc.compile()
    return nc


def _get_nc():
    if 'nc' not in _CACHE:
        _CACHE['nc'] = _build()
    return _CACHE['nc']


def make_in_maps(x, weight_q, absmax, bias):
    x = np.asarray(x, dtype=np.float32)
    weight_q = np.asarray(weight_q)
    absmax = np.asarray(absmax, dtype=np.float32)
    bias = np.asarray(bias, dtype=np.float32)

    xT = np.ascontiguousarray(x.reshape(S_TOT, IN_F).T)
    in_maps = []
    for c in range(N_CORES):
        sl = slice(c * O_LOC, (c + 1) * O_LOC)
        q_c = np.ascontiguousarray(weight_q[sl].T).astype(np.uint16)
        am = absmax[sl]                                        # [O_LOC, 64]
        scale_c = np.ascontiguousarray(
            am.T.repeat(64, axis=0)).reshape(KT, P, O_LOC)
        biasb_c = np.ascontiguousarray(np.broadcast_to(bias[sl], (P, O_LOC)))
        in_maps.append({"xT": xT, "qT": q_c, "scale": scale_c, "biasb": biasb_c})
    return in_maps


def kernel(x, weight_q, absmax, bias):
    nc = _get_nc()
    in_maps = make_in_maps(x, weight_q, absmax, bias)
    res = run_bass_kernel_spmd(nc, in_maps, core_ids=list(range(N_CORES)))
    y = np.concatenate([res.results[c]["y"] for c in range(N_CORES)], axis=1)
    return np.ascontiguousarray(y.reshape(4, 2048, OUT_F))
